# revision 71
# baseline (speedup 1.0000x reference)
"""Trainium2 Bass kernel for CrossAttention (self-attention) nn module.

Reference computation (B=2, N=4096, D=512, H=8, DH=64):
    q, k, v = x@Wq, x@Wk, x@Wv          # [B, N, 512]
    per head: S = q k^T / sqrt(64); P = softmax(S); O = P v
    out = concat_heads(O) @ Wo + bo     # [B, N, 512]

Sharding: batch*head-pair across 8 cores. Core c handles batch c//4 and
head pair c%4 (heads 2p, 2p+1). Each core computes its two heads'
attention plus its partial output projection O_pair @ Wo[rows]; the host
sums the four partials per batch (bf16 device partials) and adds bias.

Device-side strategy (per core). ScalarE exp over the 2x4096^2 score
matrix is the binding engine (~1 elem/cycle/lane @1.2GHz = 218us floor),
so everything is organized to keep the exp stream saturated:
  - Score "chunks" are 512-query x 128-key (kb, head) pairs. They stream
    through two alternating PSUM tiles, A = 3 chunks [128,1536] and
    B = 2 chunks [128,1024] (5 banks total), so each exp ACTIVATE
    covers 2.5 chunks on average: the ~172-cycle PSUM-read bubble is
    paid 26x per 512-query block instead of 32x (~9us saved).
  - S^T = K @ Q^T per head in bf16, K=64 row-packed: head0 in PE rows
    0-63, head1 in rows 64-127. No max subtraction (scores are O(1);
    exp is shift-invariant under softmax; fp32 cannot overflow here).
  - PV in bf16 with V in natural layout augmented by a ones column
    (M=65): PSUM row 64 accumulates the softmax denominator for free.
    PV lags exp by 2 score tiles, crossing qc boundaries.
  - normalize: K=1 broadcast matmul of the sum row, DVE
    reciprocal_approx_fast + multiply to bf16 O^T; output projection
    accumulates the two heads' K=64 bf16 matmuls into one PSUM bank.
    Epilogue is spread one piece per score tile (single ps_small bank);
    the last qc routes through the freed B pool + ScalarE copies.
  - Prologue: only K/Q/V of the first 512 tokens precede the exp
    stream. x^T head rides a single grouped DMA at the front of the
    Sync queue; ~4us of junk matmuls pre-warm HAM to 2.4GHz; the ACT
    table load is pulled forward by a dummy exp on a memset tile; the
    remaining K/V/Q projection blocks weave between score tiles during
    the first 512-query block (bucketed just-in-time, small bursts so
    the Tile scheduler cannot hoist them ahead of exp-gated QK pairs).
  - y written back as bf16 partials, alternating Sync/GpSimd DMA queues.

Measured on trn2 (8 cores, core-0 NTFF profile): ~297-299 us HW exec at
full clock (chip DVFS-throttles to ~1.0GHz when hot: ~360-390 us), rel
err 1.85e-3 vs the fp32 reference. Baseline before optimization: 337us.
Emission order matters: PV drains are emitted before the epilogue and
interleave groups (scheduler priority), and the previous block's
normalize/projection pieces sit at tiles 4-16, clear of the boundary.
"""

import os
import sys

import numpy as np

for _p in ("/opt/trn_rl_repo", "/root/.axon_site/_ro/trn_rl_repo"):
    if os.path.isdir(_p) and _p not in sys.path:
        sys.path.insert(0, _p)

import ml_dtypes  # noqa: E402

import concourse.bass as bass  # noqa: E402
import concourse.mybir as mybir  # noqa: E402
from concourse import bacc  # noqa: E402
from concourse.bass_utils import run_bass_kernel_spmd  # noqa: E402
from concourse.tile import TileContext  # noqa: E402

B, N, D = 2, 4096, 512
H, DH = 8, 64
P = 128                 # SBUF partitions / token block
KB = N // P             # 32 key blocks
QC = N // 512           # 8 query column blocks of 512
KCH = D // P            # 4 contraction chunks for the projections
SCALE = DH ** -0.5
NCORES = 8
K_PRE = 4               # S^T/exp steps emitted before prev qc's epilogue

# knobs for test.py
TRACE = False
LAST_RESULT = None

_CACHED_NC = None


def build_nc():
    f32 = mybir.dt.float32
    f32r = mybir.dt.float32r
    bf16 = mybir.dt.bfloat16
    Exp = mybir.ActivationFunctionType.Exp

    nc = bacc.Bacc()
    xT = nc.declare_dram_parameter("xT", [D, N], bf16, isOutput=False)
    wq = nc.declare_dram_parameter("wq", [D, P], bf16, isOutput=False)
    wk = nc.declare_dram_parameter("wk", [D, P], bf16, isOutput=False)
    wv = nc.declare_dram_parameter("wv", [D, P], bf16, isOutput=False)
    wo2_d = nc.declare_dram_parameter("wo2", [DH, 2, D], bf16, isOutput=False)
    ones_d = nc.declare_dram_parameter("ones", [P, DH], f32r, isOutput=False)
    y = nc.declare_dram_parameter("y", [N, D], bf16, isOutput=True)

    with TileContext(nc) as tc:
        with (
            tc.tile_pool(name="persist", bufs=1) as persist,
            tc.tile_pool(name="proj", bufs=1) as proj,
            tc.tile_pool(name="ptp", bufs=4) as ptp,
            tc.tile_pool(name="work", bufs=3) as work,
            tc.tile_pool(name="ps_A", bufs=1, space="PSUM") as ps_A,
            tc.tile_pool(name="ps_B", bufs=1, space="PSUM") as ps_B,
            tc.tile_pool(name="ps_acc", bufs=2, space="PSUM") as ps_acc,
            tc.tile_pool(name="ps_small", bufs=1, space="PSUM") as ps_small,
        ):
            # ---------------- prologue: loads ----------------
            # Dummy exp from a memset tile (no DMA dependency): pulls
            # ACT_TABLE_LOAD (~2.7us) off the critical path by issuing it
            # during the input DMA instead of before the first real scores.
            warm = work.tile([1, 16], f32, tag="warm")
            nc.gpsimd.memset(warm, 1.0)
            nc.scalar.activation(
                warm, warm,
                func=mybir.ActivationFunctionType.Exp, scale=1.0,
            )

            # x^T head first: the K0/Q0/V0 projections gate the whole exp
            # stream, so their 512 tokens go at the front of the DMA issue
            # queue (each dma_start costs ~650ns of Sync-engine issue time;
            # the tail of x^T goes as 3 grouped DMAs).
            xt_sb = persist.tile([P, KCH, N], bf16, tag="xt")
            xTr0 = xT.rearrange("(c p) m -> p c m", p=P)
            nc.sync.dma_start(out=xt_sb[:, :, 0:512], in_=xTr0[:, :, 0:512])

            wq_sb = persist.tile([P, KCH, P], bf16, tag="wq")
            wk_sb = persist.tile([P, KCH, P], bf16, tag="wk")
            wv_sb = persist.tile([P, KCH, P], bf16, tag="wv")
            for w_sb, w_d in ((wk_sb, wk), (wq_sb, wq), (wv_sb, wv)):
                nc.sync.dma_start(
                    out=w_sb, in_=w_d.rearrange("(c p) m -> p c m", p=P)
                )

            xTr = xT.rearrange("(c p) m -> p c m", p=P)
            for c0, c1 in ((512, 1536), (1536, 2560), (2560, 4096)):
                nc.sync.dma_start(
                    out=xt_sb[:, :, c0:c1], in_=xTr[:, :, c0:c1]
                )

            wo2_sb = persist.tile([DH, 2, D], bf16, tag="wo2")
            nc.sync.dma_start(out=wo2_sb, in_=wo2_d[:, :, :])
            ones_t = persist.tile([P, DH], f32r, tag="ones")
            nc.sync.dma_start(out=ones_t, in_=ones_d[:, :])

            # PE pre-warm: ~4us of junk matmuls while the x^T head DMA is
            # in flight, so HAM un-throttles (1.2 -> 2.4 GHz) before the
            # K0/Q0/V0 projections that gate the first exp. The junk
            # memset goes first on GpSimd so the warm-up starts early.
            junk = persist.tile([P, 512], bf16, tag="junk")
            nc.gpsimd.memset(junk, 0.0)
            psj = ps_small.tile([P, 512], f32, tag="small")
            for _ in range(10):
                nc.tensor.matmul(psj, lhsT=junk[:, 0:P], rhs=junk)

            # ones column of v_aug via GpSimd memset (a strided DMA here
            # costs ~5.6us of Sync issue time; the memset is off-queue)
            v_aug = persist.tile([P, KB, 2, DH + 1], bf16, tag="vaug")
            nc.gpsimd.memset(v_aug[:, :, :, DH:DH + 1], 1.0)

            # ---------------- projections (per 512-token block) ----------
            # QT/KT/VT: [128 (2 heads x 64 dims), 4096 tokens], bf16.
            # Only block 0 of K/Q/V runs before the attention stream starts;
            # blocks 1-7 are interleaved into qc0's steps (the PE has slack
            # while ScalarE exp is the pacer), so ScalarE starts ~30us
            # earlier than with a monolithic prologue.
            qt = persist.tile([P, N], bf16, tag="qt")
            kt = persist.tile([P, N], bf16, tag="kt")

            def emit_kqv_block(dst, w_sb, col, pool=None):
                csl = slice(col * 512, (col + 1) * 512)
                # tag "psS" when routed into a score pool: pool slots are
                # per-tag, so sharing the tag avoids an extra PSUM slot
                psp = (pool or ps_small).tile(
                    [P, 512], f32, tag="small" if pool is None else "psS"
                )
                for c in range(KCH):
                    nc.tensor.matmul(
                        psp,
                        lhsT=w_sb[:, c, :],
                        rhs=xt_sb[:, c, csl],
                        start=(c == 0),
                        stop=(c == KCH - 1),
                    )
                nc.vector.tensor_copy(dst[:, csl], psp)

            def emit_v_sub(kb, pool=None):
                # V projected straight into natural [token, dim] layout
                # (lhsT = x^T chunk, FWL-eligible): no PE transposes, no
                # intermediate vt tile. v_aug[:, kb, h, 0:64] = V block,
                # v_aug[:, kb, h, 64] = 1.0 (softmax denominator row).
                tsl = slice(kb * P, (kb + 1) * P)
                psv = (pool or ps_small).tile(
                    [P, P], f32, tag="small" if pool is None else "psS"
                )
                for c in range(KCH):
                    nc.tensor.matmul(
                        psv,
                        lhsT=xt_sb[:, c, tsl],
                        rhs=wv_sb[:, c, :],
                        start=(c == 0),
                        stop=(c == KCH - 1),
                    )
                nc.vector.tensor_copy(
                    v_aug[:, kb, :, 0:DH],
                    psv.rearrange("p (h d) -> p h d", h=2),
                )

            # prologue projections route through the (still-unused) score
            # pools so the single ps_small slot doesn't serialize them
            emit_kqv_block(kt, wk_sb, 0)
            emit_kqv_block(qt, wq_sb, 0, pool=ps_A)
            emit_v_sub(0, pool=ps_B)
            emit_v_sub(1)
            emit_v_sub(2, pool=ps_A)
            emit_v_sub(3, pool=ps_B)

            # ---------------- attention + output projection ----------------
            # Score "chunks" are 512-column (kb, head) pairs: chunk
            # c = 2*kb + h. Chunks stream through two alternating PSUM
            # score tiles, A = 3 chunks [128,1536] and B = 2 chunks
            # [128,1024], so each exp ACTIVATE covers 2.5 chunks on
            # average instead of 2: the ~180ns per-instruction PSUM-read
            # bubble is paid 26x per qc instead of 32x (~9us overall).
            state = {}
            TS = [3, 2] * 12 + [3, 1]          # 26 tiles = 64 chunks
            TSTART = [0]
            for sz in TS:
                TSTART.append(TSTART[-1] + sz)
            NT = len(TS)
            # kbs whose last chunk (2kb+1) lands in tile t
            PV_BY_TILE = [
                [(c - 1) // 2
                 for c in range(TSTART[t], TSTART[t] + TS[t]) if c % 2 == 1]
                for t in range(NT)
            ]

            def emit_tile(qc, t):
                """S^T chunks for one score tile + one fused exp."""
                qsl = slice(qc * 512, (qc + 1) * 512)
                sz, c0 = TS[t], TSTART[t]
                pool = ps_A if t % 2 == 0 else ps_B
                ps_s = pool.tile([P, sz * 512], f32, tag="psS")
                for i in range(sz):
                    kb, h = divmod(c0 + i, 2)
                    nc.tensor.matmul(
                        ps_s[:, i * 512:(i + 1) * 512],
                        lhsT=kt[h * DH:(h + 1) * DH, kb * P:(kb + 1) * P],
                        rhs=qt[h * DH:(h + 1) * DH, qsl],
                    )
                pt = ptp.tile([P, sz * 512], bf16, tag="pt")
                nc.scalar.activation(pt, ps_s, func=Exp, scale=SCALE)
                for i in range(sz):
                    state[(qc, c0 + i)] = (pt, i)

            def emit_pv(qc, kb):
                pt0, i0 = state.pop((qc, 2 * kb))
                pt1, i1 = state.pop((qc, 2 * kb + 1))
                if kb == 0:
                    state[(qc, "o0")] = ps_acc.tile(
                        [DH + 1, 512], f32, tag="psO", name="ps_o0"
                    )
                    state[(qc, "o1")] = ps_acc.tile(
                        [DH + 1, 512], f32, tag="psO", name="ps_o1"
                    )
                for h, (pt, i) in enumerate(((pt0, i0), (pt1, i1))):
                    nc.tensor.matmul(
                        state[(qc, "o0" if h == 0 else "o1")],
                        lhsT=v_aug[:, kb, h, :],
                        rhs=pt[:, i * 512:(i + 1) * 512],
                        start=(kb == 0),
                        stop=(kb == KB - 1),
                    )

            def emit_osave(qc):
                # Drain the PSUM accumulators to SBUF right away so the PV
                # slots free up ~1.5us after the last PV instead of after
                # the whole normalize chain (~3.5us).
                ps_o0 = state.pop((qc, "o0"))
                ps_o1 = state.pop((qc, "o1"))
                o0_sb = work.tile([DH + 1, 512], f32r, tag="osb")
                o1_sb = work.tile([DH + 1, 512], f32r, tag="osb")
                nc.vector.tensor_copy(o0_sb, ps_o0)
                if qc == QC - 1:
                    nc.scalar.copy(o1_sb, ps_o1)
                else:
                    nc.vector.tensor_copy(o1_sb, ps_o1)
                state[(qc, "osb")] = (o0_sb, o1_sb)

            def emit_norm_bcast(qc, h):
                # Broadcast the softmax sums (row 64) [1, 512] -> [64, 512]
                # via K=1 matmul. Split per head: the single ps_small slot
                # is shared, so the two bcasts are emitted ~2 tiles apart.
                o_sb = state[(qc, "osb")][h]
                # last qc: the exp stream is over, so the B score pool is
                # free — use it as a second slot to unserialize the tail
                if qc == QC - 1 and h == 1:
                    ps_b = ps_B.tile([DH, 512], f32, tag="psS")
                else:
                    ps_b = ps_small.tile([DH, 512], f32, tag="small")
                nc.tensor.matmul(
                    ps_b, lhsT=ones_t[DH:DH + 1, :],
                    rhs=o_sb[DH:DH + 1, :],
                )
                b_sb = work.tile([DH, 512], f32, tag="bsb")
                if qc == QC - 1:
                    # ScalarE is idle after the final exp: run the bcast
                    # drains there, in parallel with the DVE osave/recip
                    # chain that otherwise serializes the kernel tail
                    nc.scalar.copy(b_sb, ps_b)
                else:
                    nc.vector.tensor_copy(b_sb, ps_b)
                state[(qc, "bsb", h)] = b_sb

            def emit_norm_mul(qc):
                # reciprocal then normalize to bf16 O^T per head.
                # (reciprocal_approx_fast silently returns zeros when fed
                # PSUM on HW, so it runs SBUF->SBUF after the bcast.)
                o0_sb, o1_sb = state.pop((qc, "osb"))
                otn0 = work.tile([DH, 512], bf16, tag="otn")
                otn1 = work.tile([DH, 512], bf16, tag="otn")
                b0_sb = state.pop((qc, "bsb", 0))
                b1_sb = state.pop((qc, "bsb", 1))
                binv0 = work.tile([DH, 512], f32, tag="binv")
                binv1 = work.tile([DH, 512], f32, tag="binv")
                nc.vector.reciprocal_approx_fast(out=binv0, in_=b0_sb)
                nc.vector.reciprocal_approx_fast(out=binv1, in_=b1_sb)
                nc.vector.tensor_mul(otn0, o0_sb[0:DH, :], binv0)
                nc.vector.tensor_mul(otn1, o1_sb[0:DH, :], binv1)
                state[(qc, "otn")] = (otn0, otn1)

            def emit_proj_sub(qc, sub):
                # output projection: y[q, :] = sum_h O_h[q, :] @ Wo_h,
                # one 128-row sub-block at a time (shared ps_small slot)
                otn0, otn1 = state[(qc, "otn")]
                ssl = slice(sub * P, (sub + 1) * P)
                if qc == QC - 1 and sub % 2 == 1:
                    ps_y = ps_B.tile([P, 512], f32, tag="psS")
                else:
                    ps_y = ps_small.tile([P, 512], f32, tag="small")
                nc.tensor.matmul(
                    ps_y, lhsT=otn0[:, ssl], rhs=wo2_sb[:, 0, :],
                    start=True, stop=False,
                )
                nc.tensor.matmul(
                    ps_y, lhsT=otn1[:, ssl], rhs=wo2_sb[:, 1, :],
                    start=False, stop=True,
                )
                ysb = work.tile([P, 512], bf16, tag="ysb")
                nc.vector.tensor_copy(ysb, ps_y)
                r0 = qc * 512 + sub * P
                # alternate output queues (Sync HW / GpSimd SW DGE) so
                # y writebacks don't contend with the x^T input stream
                eng = nc.sync if sub % 2 == 0 else nc.gpsimd
                eng.dma_start(out=y[r0:r0 + P, :], in_=ysb)
                if sub == 3:
                    state.pop((qc, "otn"))

            # qc0 interleave buckets: chunk->tile map drives due dates.
            CT = [None] * (TSTART[-1])
            for t in range(NT):
                for c in range(TSTART[t], TSTART[t] + TS[t]):
                    CT[c] = t
            buckets = {t: [] for t in range(NT)}
            for b in range(1, 8):          # K blocks, due at chunk 8b
                buckets[max(1, CT[8 * b] - 4)].append(("k", b))
            for kbv in range(4, KB):       # V subs, due at PV(kbv)
                buckets[max(1, CT[2 * kbv + 1] - 2)].append(("v", kbv))

            # Software pipeline, flat over all 8*26 score tiles so the
            # 2-tile PV lag crosses qc boundaries. norm/proj of qc-1 are
            # spread over tiles 2..14 of qc (the single ps_small slot is
            # reused every other tile). During qc0, K blocks and single
            # 128-token V sub-blocks weave between tiles just before
            # QK/PV first needs them; Q for qc+1 is emitted mid-qc.
            for gt in range(QC * NT + 2):
                if gt < QC * NT:
                    qc, t = divmod(gt, NT)
                    emit_tile(qc, t)
                if gt >= 2:
                    pqc, pt_ = divmod(gt - 2, NT)
                    for kb in PV_BY_TILE[pt_]:
                        emit_pv(pqc, kb)
                        if kb == KB - 1:
                            emit_osave(pqc)
                if gt < QC * NT:
                    # epilogue/interleave after the PV drain: lag-critical
                    # work gets earlier scheduler priority than the
                    # hoistable projection/normalize groups
                    if qc > 0:
                        if t == 6:
                            emit_norm_bcast(qc - 1, 0)
                        elif t == 8:
                            emit_norm_bcast(qc - 1, 1)
                        elif t == 10:
                            emit_norm_mul(qc - 1)
                        elif t in (12, 14, 16, 18):
                            emit_proj_sub(qc - 1, (t - 12) // 2)
                    if qc == 0:
                        for kind, arg in buckets[t]:
                            if kind == "k":
                                emit_kqv_block(kt, wk_sb, arg)
                            else:
                                emit_v_sub(arg)
                    if qc < QC - 1 and t == 20:
                        emit_kqv_block(qt, wq_sb, qc + 1)
            emit_norm_bcast(QC - 1, 0)
            emit_norm_bcast(QC - 1, 1)
            emit_norm_mul(QC - 1)
            for sub in range(4):
                emit_proj_sub(QC - 1, sub)

    if not nc.is_finalized():
        nc.finalize()
    return nc


def _get_nc():
    global _CACHED_NC
    if _CACHED_NC is None:
        _CACHED_NC = build_nc()
    return _CACHED_NC


def make_in_maps(x, Wq, Wk, Wv, Wo):
    in_maps = []
    for c in range(NCORES):
        b, p = c // 4, c % 4
        cols = slice(p * P, (p + 1) * P)
        wo2 = (
            Wo[cols, :]
            .reshape(2, DH, D)
            .transpose(1, 0, 2)
            .astype(ml_dtypes.bfloat16)
        )
        in_maps.append({
            "xT": np.ascontiguousarray(x[b].T).astype(ml_dtypes.bfloat16),
            "wq": np.ascontiguousarray(Wq[:, cols]).astype(ml_dtypes.bfloat16),
            "wk": np.ascontiguousarray(Wk[:, cols]).astype(ml_dtypes.bfloat16),
            "wv": np.ascontiguousarray(Wv[:, cols]).astype(ml_dtypes.bfloat16),
            "wo2": np.ascontiguousarray(wo2),
            "ones": np.ones((P, DH), dtype=np.float32),  # f32r param, same bits
        })
    return in_maps


def kernel(x, Wq, Wk, Wv, Wo, bo):
    global LAST_RESULT
    x = np.asarray(x, dtype=np.float32)
    Wq = np.asarray(Wq, dtype=np.float32)
    Wk = np.asarray(Wk, dtype=np.float32)
    Wv = np.asarray(Wv, dtype=np.float32)
    Wo = np.asarray(Wo, dtype=np.float32)
    bo = np.asarray(bo, dtype=np.float32)

    in_maps = make_in_maps(x, Wq, Wk, Wv, Wo)
    nc = _get_nc()
    res = run_bass_kernel_spmd(nc, in_maps, list(range(NCORES)), trace=TRACE)
    LAST_RESULT = res

    out = np.zeros((B, N, D), dtype=np.float32)
    for c in range(NCORES):
        out[c // 4] += res.results[c]["y"].astype(np.float32)
    out += bo[None, None, :]
    return out

